# revision 2
# baseline (speedup 1.0000x reference)
"""GAT layer kernel for Trainium2, 8 NeuronCores, data-parallel over batch.

Math (per batch b):
    Wh = x @ W1.T + b1
    e_l = Wh @ a_w[:D]; e_r = Wh @ a_w[D:]
    s = LeakyReLU_{0.1}(e_l[i] + e_r[j] + a_b) + pos[b, i, j]
    w = softmax_j(s);  out = ELU(w @ Wh)

Device computes, per batch (in transposed [j, i] layout):
    Wh0 = x @ W1.T                       (no bias; bias folds out, see below)
    er[j] = x @ (W1.T @ a_r)             el'[i] = x @ (W1.T @ a_l) + c
        where c = b1@a_l + b1@a_r + a_b  (so el' + er == e_l + e_r + a_b)
    pre[j, i] = er[j] + el'[i]           (rank-2 matmul trick)
    expT[j, i] = exp(Prelu_0.1(pre) + posT)
    U[i, d], den[i] = expT.T @ [Wh0 | 1] (ones column gives the softmax denom)
Host epilogue:  out = ELU(U/den + b1)
    (exact: softmax @ (Wh0 + 1 b1^T) = (exp@Wh0)/den + b1 since exp@1 = den)

Matmuls run in float32r (TF32-like, 1 cyc/row); ~2e-4 absmax error total.
"""
import sys

sys.path.insert(0, "/opt/trn_rl_repo")

import numpy as np

import bass_rust as _bass_rust
import concourse.bass as bass
import concourse.bacc as bacc
import concourse.tile as tile_mod
from concourse import mybir
from concourse.bass_utils import run_bass_kernel_spmd

# ---------------------------------------------------------------------------
# Workaround: this container's walrus rejects >1 sync wait on a tail Drain.
# Split the TileContext tail-drain waits across one Drain per producing proc.
def _split_drain_and_barrier(self, tick_clock, wait_clock):
    gc = list(tick_clock.global_clock)
    for proc, sem in sorted(self.sems.allocated().items()):
        if gc[proc] <= 0:
            continue
        vec = [0] * len(gc)
        vec[proc] = gc[proc]
        drain_inst = self.nc.sync.drain()
        wait_clock.add_sem_waits(
            drain_inst.ins,
            _bass_rust.ScopedClock({None: _bass_rust.VectorClock(vec)}),
        )
    self.nc.sync.drain()
    self.nc.all_engine_barrier()
    assert self.sems is not None
    popped = self.nc._tile_sem_poison_stack.pop()
    assert popped is self._sem_poison
    self.nc.clear_and_free_semaphores(list(self.sems.allocated().values()))
    self.nc.all_engine_barrier()


tile_mod.TileContext._drain_and_barrier = _split_drain_and_barrier
# ---------------------------------------------------------------------------

F32 = mybir.dt.float32
F32R = mybir.dt.float32r
AF = mybir.ActivationFunctionType
ALU = mybir.AluOpType

B, S, D = 512, 256, 256
NCORES = 8
BLOC = B // NCORES          # 64 batches per core
G = BLOC // 2               # 32 groups of 2 batches
NEG_SLOPE = 0.1
NAUG = 258                  # Wh cols + ones col + zero pad (N must be even)

_PROG = None


def _build_program():
    nc = bacc.Bacc()
    xT = nc.declare_dram_parameter("xT", [G, 128, 2, 2, 256], F32, isOutput=False)
    posT = nc.declare_dram_parameter("posT", [G, 128, 2, 2, 256], F32, isOutput=False)
    w1T = nc.declare_dram_parameter("w1T", [128, 2, 256], F32, isOutput=False)
    u2 = nc.declare_dram_parameter("u2", [128, 2, 2], F32, isOutput=False)
    cst = nc.declare_dram_parameter("cst", [1, 1], F32, isOutput=False)
    init = nc.declare_dram_parameter("init", [128, 1032], F32, isOutput=False)
    out_r = nc.declare_dram_parameter("out_r", [G, 2, 128, 2, NAUG], F32, isOutput=True)

    with tile_mod.TileContext(nc) as tc:
        with (
            tc.tile_pool(name="consts", bufs=1) as consts,
            tc.tile_pool(name="xp", bufs=3) as xp,
            tc.tile_pool(name="pp", bufs=3) as pp,
            tc.tile_pool(name="s1p", bufs=2) as s1p,
            tc.tile_pool(name="s2p", bufs=2) as s2p,
            tc.tile_pool(name="exq", bufs=2) as exq,
            tc.tile_pool(name="outp", bufs=4) as outp,
            tc.tile_pool(name="mm", bufs=3, space="PSUM") as mmp,
            tc.tile_pool(name="prep", bufs=1, space="PSUM") as prep,
        ):
            w1s = consts.tile([128, 2, 256], F32R)
            u2s = consts.tile([128, 2, 2], F32R)
            cs = consts.tile([1, 1], F32)
            nc.sync.dma_start(out=w1s[:], in_=w1T.ap()[:].bitcast(F32R))
            nc.sync.dma_start(out=u2s[:], in_=u2.ap()[:].bitcast(F32R))
            nc.sync.dma_start(out=cs[:], in_=cst.ap()[:])

            # Static rank-2 operand tiles (double-buffered manually):
            #   A [33, b2, 128-j-cols]: row0 = er (per group), rows 1..32 = 1.0
            #   Bt [33, b2, 256-i-cols]: row0 = 1.0, rows 1..31 = 0, row32 = el'
            # pre = A.T @ Bt = er[j] + el'[i]
            A_t = [consts.tile([33, 2, 256], F32R, tag=f"A{i}", name=f"A{i}") for i in range(2)]
            B_t = [consts.tile([33, 2, 256], F32R, tag=f"B{i}", name=f"B{i}") for i in range(2)]
            wha = [consts.tile([128, 4, NAUG], F32R, tag=f"wha{i}", name=f"wha{i}") for i in range(2)]
            for i in range(2):
                nc.sync.dma_start(out=A_t[i][:], in_=init.ap()[0:33, 0:512].bitcast(F32R))
                nc.sync.dma_start(out=B_t[i][:], in_=init.ap()[0:33, 512:1024].bitcast(F32R))
                nc.sync.dma_start(
                    out=wha[i][:, :, 256:NAUG], in_=init.ap()[:, 1024:1032].bitcast(F32R)
                )

            for g in range(G):
                xg = xp.tile([128, 2, 2, 256], F32R, tag="xg")       # [dp, k, b2, s]
                nc.sync.dma_start(out=xg[:], in_=xT.ap()[g].bitcast(F32R))
                posg = pp.tile([128, 2, 2, 256], F32, tag="posg")    # [jp, b2, jt, i]
                nc.sync.dma_start(out=posg[:], in_=posT.ap()[g])

                A, Bt, whaug = A_t[g % 2], B_t[g % 2], wha[g % 2]

                # Wh matmuls: wh_ps[:, b2*2+st, :] = x(b)[s-tile].T-block @ W1.T
                wh_ps = mmp.tile([128, 4, 256], F32, tag="mm")
                for b2 in range(2):
                    for st in range(2):
                        for k in range(2):
                            nc.tensor.matmul(
                                wh_ps[:, b2 * 2 + st, :],
                                xg[:, k, b2, bass.ts(st, 128)],
                                w1s[:, k, :],
                                start=(k == 0),
                                stop=(k == 1),
                            )

                # e matmuls (M=1): er -> pre[0, cols 0:512], el -> pre[0, cols 512:1024]
                pre_ps = prep.tile([128, 4, 256], F32, tag="pre")
                for which in range(2):  # 0 = r, 1 = l
                    for k in range(2):
                        nc.tensor.matmul(
                            pre_ps[0:1, 2 * which : 2 * which + 2, :],
                            u2s[:, k, which : which + 1],
                            xg[:, k, :, :],
                            start=(k == 0),
                            stop=(k == 1),
                        )
                # er -> A row0 ; el' = el + c -> Bt row32
                nc.vector.tensor_copy(A[0:1, :, :], pre_ps[0:1, 0:2, :])
                nc.vector.tensor_scalar(
                    B_t[g % 2][32:33, :, :], pre_ps[0:1, 2:4, :], cs[0:1, 0:1], None, ALU.add
                )

                # rank-2: pre[:, b2*2+jt, :] = er[j-tile] + el'[i]
                for b2 in range(2):
                    for jt in range(2):
                        nc.tensor.matmul(
                            pre_ps[:, b2 * 2 + jt, :],
                            A[:, b2, bass.ts(jt, 128)],
                            Bt[:, b2, :],
                            start=True,
                            stop=True,
                        )

                # LeakyReLU(0.1) -> +pos -> exp
                s1 = s1p.tile([128, 4, 256], F32, tag="s1")
                nc.scalar.activation(s1[:], pre_ps[:, :, :], AF.Prelu, alpha=NEG_SLOPE)
                s2 = s2p.tile([128, 4, 256], F32, tag="s2")
                nc.gpsimd.tensor_tensor(s2[:], s1[:], posg[:], op=ALU.add)
                expg = exq.tile([128, 4, 256], F32R, tag="expg")
                nc.scalar.activation(expg[:], s2[:], AF.Exp)

                # Wh -> SBUF (f32r) into whaug cols 0:256 (ones col is static)
                nc.vector.tensor_copy(whaug[:, :, 0:256], wh_ps[:, :, :])

                # agg matmuls + copies out
                for b2 in range(2):
                    agg_ps = mmp.tile([128, 2, 512], F32, tag="mm")
                    for it in range(2):
                        for jt in range(2):
                            nc.tensor.matmul(
                                agg_ps[:, it, 0:NAUG],
                                expg[:, b2 * 2 + jt, bass.ts(it, 128)],
                                whaug[:, b2 * 2 + jt, :],
                                start=(jt == 0),
                                stop=(jt == 1),
                            )
                    outsb = outp.tile([128, 2, NAUG], F32, tag="outsb")
                    if b2 == 0:
                        nc.scalar.copy(outsb[:, :, :], agg_ps[:, :, 0:NAUG])
                    else:
                        nc.vector.tensor_copy(outsb[:, :, :], agg_ps[:, :, 0:NAUG])
                    nc.sync.dma_start(out=out_r.ap()[g, b2], in_=outsb[:])

    nc.finalize()
    return nc


def _get_prog():
    global _PROG
    if _PROG is None:
        _PROG = _build_program()
    return _PROG


def _host_inputs(x, pos_matrix, W1, b1, a_w, a_b):
    """Build per-core input maps."""
    x = np.asarray(x, dtype=np.float32)
    pos = np.asarray(pos_matrix, dtype=np.float32)[..., 0]  # [B, i, j]
    W1 = np.asarray(W1, dtype=np.float32)
    b1 = np.asarray(b1, dtype=np.float32)
    a_w = np.asarray(a_w, dtype=np.float32)
    a_b = np.asarray(a_b, dtype=np.float32)

    a_l, a_r = a_w[:D], a_w[D:]
    u_l = W1.T @ a_l
    u_r = W1.T @ a_r
    c = np.float32(b1 @ a_l + b1 @ a_r + a_b[0])

    w1T_v = np.ascontiguousarray(W1.T.reshape(2, 128, 256).transpose(1, 0, 2))
    u2_v = np.ascontiguousarray(
        np.stack([u_r.reshape(2, 128), u_l.reshape(2, 128)], axis=-1).transpose(1, 0, 2)
    )
    cst_v = np.array([[c]], dtype=np.float32)
    init_v = np.zeros((128, 1032), dtype=np.float32)
    init_v[0:33, 0:512] = 1.0              # A: all ones
    init_v[0, 512:1024] = 1.0              # B: row0 ones, rest zero
    init_v[:, 1024:1032] = np.tile(np.array([1.0, 0.0], np.float32), 4)  # whaug cols

    in_maps = []
    for cix in range(NCORES):
        sl = slice(cix * BLOC, (cix + 1) * BLOC)
        xl = x[sl]                          # [64, s, d]
        # xT_r[g, p, k, b2, s] = xl[2g+b2, s, 128k+p]
        xT_v = np.ascontiguousarray(
            xl.reshape(G, 2, S, 2, 128).transpose(0, 4, 3, 1, 2)
        )
        pl = pos[sl]                        # [64, i, j]
        # posT_r[g, p, b2, jt, i] = pl[2g+b2, i, 128jt+p]
        posT_v = np.ascontiguousarray(
            pl.reshape(G, 2, S, 2, 128).transpose(0, 4, 1, 3, 2)
        )
        in_maps.append(
            {
                "xT": xT_v,
                "posT": posT_v,
                "w1T": w1T_v,
                "u2": u2_v,
                "cst": cst_v,
                "init": init_v,
            }
        )
    return in_maps, b1


def _host_epilogue(results, b1):
    out = np.empty((B, S, D), dtype=np.float32)
    for cix in range(NCORES):
        r = results[cix]["out_r"]           # [G, 2, 128, 2, NAUG]
        U = r[..., 0:256]                   # [G, b2, p, it, d]
        den = r[..., 256]                   # [G, b2, p, it]
        # batch = cix*BLOC + 2g + b2 ; i = 128*it + p
        a = U / den[..., None] + b1
        # -> [G, b2, it, p, d] -> [64, 256, 256]
        a = a.transpose(0, 1, 3, 2, 4).reshape(BLOC, S, D)
        out[cix * BLOC : (cix + 1) * BLOC] = np.where(a > 0, a, np.expm1(a))
    return out


def kernel(x, pos_matrix, W1, b1, a_w, a_b):
    nc = _get_prog()
    in_maps, b1v = _host_inputs(x, pos_matrix, W1, b1, a_w, a_b)
    res = run_bass_kernel_spmd(nc, in_maps, core_ids=list(range(NCORES)))
    return _host_epilogue(res.results, b1v)


# revision 14
# speedup vs baseline: 1.1181x; 1.1181x over previous
"""GAT layer kernel for Trainium2, 8 NeuronCores, data-parallel over batch.

Math (per batch b):
    Wh = x @ W1.T + b1
    e_l = Wh @ a_w[:D]; e_r = Wh @ a_w[D:]
    s = LeakyReLU_{0.1}(e_l[i] + e_r[j] + a_b) + pos[b, i, j]
    w = softmax_j(s);  out = ELU(w @ Wh)

Device computes, per batch (in transposed [j, i] layout):
    Wh0 = x @ W1.T                       (no bias; bias folds out, see below)
    er[j] = x @ (W1.T @ a_r)             el'[i] = x @ (W1.T @ a_l) + c
        where c = b1@a_l + b1@a_r + a_b  (so el' + er == e_l + e_r + a_b)
    pre[j, i] = er[j] + el'[i]           (rank-2 matmul trick)
    expT[j, i] = exp(Prelu_0.1(pre) + posT)
    U[i, d], den[i] = expT.T @ [Wh0 | 1] (ones column gives the softmax denom)
Host epilogue:  out = ELU(U/den + b1)
    (exact: softmax @ (Wh0 + 1 b1^T) = (exp@Wh0)/den + b1 since exp@1 = den)

Heavy matmuls run in float32r (TF32-like, 1 cyc/row). x/pos/U I/O can run
in bf16 (flags below) to cut HBM traffic; den stays fp32.
"""
import sys

sys.path.insert(0, "/opt/trn_rl_repo")

import numpy as np
import ml_dtypes

import bass_rust as _bass_rust
import concourse.bass as bass
import concourse.bacc as bacc
import concourse.tile as tile_mod
from concourse import mybir
from concourse.bass_utils import run_bass_kernel_spmd

# ---------------------------------------------------------------------------
# Workaround: this container's walrus rejects >1 sync wait on a tail Drain.
# Split the TileContext tail-drain waits across one Drain per producing proc.
def _split_drain_and_barrier(self, tick_clock, wait_clock):
    gc = list(tick_clock.global_clock)
    for proc, sem in sorted(self.sems.allocated().items()):
        if gc[proc] <= 0:
            continue
        vec = [0] * len(gc)
        vec[proc] = gc[proc]
        drain_inst = self.nc.sync.drain()
        wait_clock.add_sem_waits(
            drain_inst.ins,
            _bass_rust.ScopedClock({None: _bass_rust.VectorClock(vec)}),
        )
    self.nc.sync.drain()
    self.nc.all_engine_barrier()
    assert self.sems is not None
    popped = self.nc._tile_sem_poison_stack.pop()
    assert popped is self._sem_poison
    self.nc.clear_and_free_semaphores(list(self.sems.allocated().values()))
    self.nc.all_engine_barrier()


tile_mod.TileContext._drain_and_barrier = _split_drain_and_barrier
# ---------------------------------------------------------------------------

F32 = mybir.dt.float32
F32R = mybir.dt.float32r
BF16 = mybir.dt.bfloat16
AF = mybir.ActivationFunctionType
ALU = mybir.AluOpType

B, S, D = 512, 256, 256
NCORES = 8
BLOC = B // NCORES          # 64 batches per core
G = BLOC // 2               # 32 groups of 2 batches
NEG_SLOPE = 0.1
NAUG = 258
NBUF = 3                    # manual buffering depth for A/B/whaug statics                  # Wh cols + ones col + zero pad (N must be even)

X_BF16 = True               # ship x/W1/u2 in bf16 (Wh & e matmuls in bf16)
POS_BF16 = True             # ship pos in bf16 (added to fp32 logits on device)
OUT_BF16 = True             # ship U in bf16 (den stays fp32)

_PROG = None


def _build_program():
    xdt = BF16 if X_BF16 else F32
    xmmdt = BF16 if X_BF16 else F32R
    pdt = BF16 if POS_BF16 else F32
    odt = BF16 if OUT_BF16 else F32

    assert X_BF16 == POS_BF16, "merged xpos DMA needs matching dtypes"
    nc = bacc.Bacc()
    xpos = nc.declare_dram_parameter("xpos", [G, 128, 2, 2, 2, 256], xdt, isOutput=False)
    w1T = nc.declare_dram_parameter("w1T", [128, 2, 256], xdt, isOutput=False)
    u2 = nc.declare_dram_parameter("u2", [128, 2, 2], xdt, isOutput=False)
    cst = nc.declare_dram_parameter("cst", [2, 1], F32, isOutput=False)
    init = nc.declare_dram_parameter("init", [128, 1032], F32, isOutput=False)
    out_r = nc.declare_dram_parameter("out_r", [G, 128, 2, 2, 256], odt, isOutput=True)
    den_r = nc.declare_dram_parameter("den_r", [128, G, 2, 2], F32, isOutput=True)

    def mmcast(ap):
        return ap if X_BF16 else ap.bitcast(F32R)

    with tile_mod.TileContext(nc) as tc:
        with (
            tc.tile_pool(name="consts", bufs=1) as consts,
            tc.tile_pool(name="xp", bufs=6) as xp,
            tc.tile_pool(name="pp", bufs=4) as pp,
            tc.tile_pool(name="s1p", bufs=4) as s1p,
            tc.tile_pool(name="s2p", bufs=4) as s2p,
            tc.tile_pool(name="exq", bufs=4) as exq,
            tc.tile_pool(name="outp", bufs=6) as outp,
            tc.tile_pool(name="mm", bufs=2, space="PSUM") as mmp,
            tc.tile_pool(name="prep", bufs=2, space="PSUM") as prep,
        ):
            prefetch = {}
            xpgp0 = xp.tile([128, 2, 2, 2, 256], xmmdt, tag="xg", name="xpgpre0")
            nc.sync.dma_start(out=xpgp0[:], in_=mmcast(xpos.ap()[0]))
            prefetch[0] = xpgp0
            w1s = consts.tile([128, 2, 256], xmmdt)
            u2s = consts.tile([128, 2, 2], xmmdt)
            cs = consts.tile([2, 1], F32)
            nc.sync.dma_start(out=w1s[:], in_=mmcast(w1T.ap()[:]))
            nc.sync.dma_start(out=u2s[:], in_=mmcast(u2.ap()[:]))
            nc.sync.dma_start(out=cs[:], in_=cst.ap()[:])

            # Static rank-2 operand tiles (double-buffered manually):
            #   A [33, b2, 128-j-cols]: row0 = er (per group), rows 1..32 = 1.0
            #   Bt [33, b2, 256-i-cols]: row0 = 1.0, rows 1..31 = 0, row32 = el'
            # pre = A.T @ Bt = er[j] + el'[i]
            A_t = [consts.tile([34, 2, 256], F32R, tag=f"A{i}", name=f"A{i}") for i in range(NBUF)]
            B_t = [consts.tile([34, 2, 256], F32R, tag=f"B{i}", name=f"B{i}") for i in range(NBUF)]
            wha = [consts.tile([128, 4, NAUG], F32R, tag=f"wha{i}", name=f"wha{i}") for i in range(NBUF)]
            def _init_statics(i):
                nc.sync.dma_start(out=A_t[i][:], in_=init.ap()[0:34, 0:512].bitcast(F32R))
                nc.sync.dma_start(out=B_t[i][:], in_=init.ap()[0:34, 512:1024].bitcast(F32R))
                nc.sync.dma_start(
                    out=wha[i][:, :, 256:NAUG], in_=init.ap()[:, 1024:1032].bitcast(F32R)
                )
            _init_statics(0)
            xpgp1 = xp.tile([128, 2, 2, 2, 256], xmmdt, tag="xg", name="xpgpre1")
            nc.sync.dma_start(out=xpgp1[:], in_=mmcast(xpos.ap()[1]))
            prefetch[1] = xpgp1
            for i in range(1, NBUF):
                _init_statics(i)

            denall = consts.tile([128, G, 2, 2], F32)

            # ---- software-pipelined group loop (1-group skew) ----------------
            # stage1(g): loads + all PE front work (Wh, e, rank-2) + copies
            # stage2(g): lrelu
            # stage3(g): posadd, exp, agg matmuls, output copies + DMA
            # Emission order s1(g), s2(g), s3(g-1) lets ACT run lrelu(g) while
            # POOL does posadd(g-1), hiding the ACT->POOL->ACT ping-pong.
            xpgs, preps, s1s, whs = {}, {}, {}, {}

            def stage1(g):
                if g in prefetch:
                    xpg = prefetch[g]
                else:
                    xpg = xp.tile([128, 2, 2, 2, 256], xmmdt, tag="xg", name=f"xpg{g}")
                    nc.sync.dma_start(out=xpg[:], in_=mmcast(xpos.ap()[g]))
                xpgs[g] = xpg
                xg = xpg[:, 0]
                A, Bt, whaug = A_t[g % NBUF], B_t[g % NBUF], wha[g % NBUF]

                wh_ps = mmp.tile([128, 4, 256], F32, tag="mm", name=f"wh{g}")
                whs[g] = wh_ps
                for b2 in range(2):
                    for st in range(2):
                        for k in range(2):
                            nc.tensor.matmul(
                                wh_ps[:, b2 * 2 + st, :],
                                xg[:, k, b2, bass.ts(st, 128)],
                                w1s[:, k, :],
                                start=(k == 0),
                                stop=(k == 1),
                            )
                pre_ps = prep.tile([128, 4, 256], F32, tag="pre", name=f"pre{g}")
                preps[g] = pre_ps
                for k in range(2):
                    nc.tensor.matmul(
                        pre_ps[0:2, 0:2, :],
                        u2s[:, k, :],
                        xg[:, k, :, :],
                        start=(k == 0),
                        stop=(k == 1),
                    )
                nc.vector.tensor_copy(A[0:2, :, :], pre_ps[0:2, 0:2, :])
                nc.vector.tensor_scalar(
                    Bt[32:34, :, :], pre_ps[0:2, 0:2, :], cs[0:2, 0:1], None, ALU.add
                )
                for b2 in range(2):
                    for jt in range(2):
                        nc.tensor.matmul(
                            pre_ps[:, b2 * 2 + jt, :],
                            A[:, b2, bass.ts(jt, 128)],
                            Bt[:, b2, :],
                            start=True,
                            stop=True,
                        )
                nc.vector.tensor_copy(whaug[:, :, 0:256], wh_ps[:, :, :])

            def stage2(g):
                s1 = s1p.tile([128, 4, 256], F32, tag="s1", name=f"s1_{g}")
                nc.scalar.activation(s1[:], preps[g][:, :, :], AF.Prelu, alpha=NEG_SLOPE)
                s1s[g] = s1

            def stage3(g):
                whaug = wha[g % NBUF]
                posg = xpgs[g][:, 1]
                s2 = s2p.tile([128, 4, 256], F32, tag="s2", name=f"s2_{g}")
                nc.gpsimd.tensor_tensor(s2[:], s1s[g][:], posg[:], op=ALU.add)
                expg = exq.tile([128, 4, 256], F32R, tag="expg", name=f"ex{g}")
                nc.scalar.activation(expg[:], s2[:], AF.Exp)

                outsb = outp.tile([128, 2, 2, 256], odt, tag="outsb", name=f"osb{g}")
                for b2 in range(2):
                    agg_ps = mmp.tile([128, 2, 512], F32, tag="mm", name=f"agg{g}_{b2}")
                    for it in range(2):
                        for jt in range(2):
                            nc.tensor.matmul(
                                agg_ps[:, it, 0:NAUG],
                                expg[:, b2 * 2 + jt, bass.ts(it, 128)],
                                whaug[:, b2 * 2 + jt, :],
                                start=(jt == 0),
                                stop=(jt == 1),
                            )
                    if b2 == 0 or (g % 2 == 1):
                        nc.scalar.copy(outsb[:, b2, :, :], agg_ps[:, :, 0:256])
                    else:
                        nc.vector.tensor_copy(outsb[:, b2, :, :], agg_ps[:, :, 0:256])
                    nc.vector.tensor_copy(denall[:, g, b2, :], agg_ps[:, :, 256])
                nc.sync.dma_start(out=out_r.ap()[g], in_=outsb[:])
                del xpgs[g], preps[g], s1s[g], whs[g]

            for g in range(G + 1):
                if g < G:
                    stage1(g)
                    stage2(g)
                if g > 0:
                    stage3(g - 1)
            nc.sync.dma_start(out=den_r.ap()[:], in_=denall[:])

    nc.finalize()
    return nc


def _get_prog():
    global _PROG
    if _PROG is None:
        _PROG = _build_program()
    return _PROG


def _host_inputs(x, pos_matrix, W1, b1, a_w, a_b):
    """Build per-core input maps."""
    x = np.asarray(x, dtype=np.float32)
    pos = np.asarray(pos_matrix, dtype=np.float32)[..., 0]  # [B, i, j]
    W1 = np.asarray(W1, dtype=np.float32)
    b1 = np.asarray(b1, dtype=np.float32)
    a_w = np.asarray(a_w, dtype=np.float32)
    a_b = np.asarray(a_b, dtype=np.float32)

    xdt = ml_dtypes.bfloat16 if X_BF16 else np.float32
    pdt = ml_dtypes.bfloat16 if POS_BF16 else np.float32

    a_l, a_r = a_w[:D], a_w[D:]
    u_l = W1.T @ a_l
    u_r = W1.T @ a_r
    c = np.float32(b1 @ a_l + b1 @ a_r + a_b[0])

    w1T_v = np.ascontiguousarray(W1.T.reshape(2, 128, 256).transpose(1, 0, 2)).astype(xdt)
    u2_v = np.ascontiguousarray(
        np.stack([u_r.reshape(2, 128), u_l.reshape(2, 128)], axis=-1).transpose(1, 0, 2)
    ).astype(xdt)
    cst_v = np.array([[c], [c]], dtype=np.float32)
    init_v = np.zeros((128, 1032), dtype=np.float32)
    init_v[0:34, 0:512] = 1.0              # A: ones ...
    init_v[32, 0:512] = 0.0                # ... except row 32 (kills er+c in B row32)
    init_v[0, 512:1024] = 1.0              # B: row0 ones, rest zero
    init_v[:, 1024:1032] = np.tile(np.array([1.0, 0.0], np.float32), 4)  # whaug cols

    in_maps = []
    for cix in range(NCORES):
        sl = slice(cix * BLOC, (cix + 1) * BLOC)
        xl = x[sl]                          # [64, s, d]
        # xT[g, p, k, b2, s] = xl[2g+b2, s, 128k+p]
        xT_v = xl.reshape(G, 2, S, 2, 128).transpose(0, 4, 3, 1, 2)
        pl = pos[sl]                        # [64, i, j]
        # posT[g, p, b2, jt, i] = pl[2g+b2, i, 128jt+p]
        posT_v = pl.reshape(G, 2, S, 2, 128).transpose(0, 4, 1, 3, 2)
        xpos_v = np.empty((G, 128, 2, 2, 2, 256), dtype=xdt)
        xpos_v[:, :, 0] = xT_v
        xpos_v[:, :, 1] = posT_v
        in_maps.append(
            {
                "xpos": xpos_v,
                "w1T": w1T_v,
                "u2": u2_v,
                "cst": cst_v,
                "init": init_v,
            }
        )
    return in_maps, b1


def _host_epilogue(results, b1):
    out = np.empty((B, S, D), dtype=np.float32)
    for cix in range(NCORES):
        U = np.asarray(results[cix]["out_r"], dtype=np.float32)   # [G, p, b2, it, d]
        den = np.asarray(results[cix]["den_r"], dtype=np.float32) # [p, G, b2, it]
        den = den.transpose(1, 0, 2, 3)                            # [G, p, b2, it]
        a = U / den[..., None] + b1
        # [G, p, b2, it, d] -> [G, b2, it, p, d] -> [64, 256, 256]
        a = a.transpose(0, 2, 3, 1, 4).reshape(BLOC, S, D)
        out[cix * BLOC : (cix + 1) * BLOC] = np.where(a > 0, a, np.expm1(a))
    return out


def kernel(x, pos_matrix, W1, b1, a_w, a_b):
    nc = _get_prog()
    in_maps, b1v = _host_inputs(x, pos_matrix, W1, b1, a_w, a_b)
    res = run_bass_kernel_spmd(nc, in_maps, core_ids=list(range(NCORES)))
    return _host_epilogue(res.results, b1v)


# revision 17
# speedup vs baseline: 916.9633x; 820.1268x over previous
"""GAT layer kernel for Trainium2, 8 NeuronCores, data-parallel over batch.

Math (per batch b):
    Wh = x @ W1.T + b1
    e_l = Wh @ a_w[:D]; e_r = Wh @ a_w[D:]
    s = LeakyReLU_{0.1}(e_l[i] + e_r[j] + a_b) + pos[b, i, j]
    w = softmax_j(s);  out = ELU(w @ Wh)

Device computes, per batch (in transposed [j, i] layout):
    Wh0 = x @ W1.T                       (no bias; bias folds out, see below)
    er[j] = x @ (W1.T @ a_r)             el'[i] = x @ (W1.T @ a_l) + c
        where c = b1@a_l + b1@a_r + a_b  (so el' + er == e_l + e_r + a_b)
    pre[j, i] = er[j] + el'[i]           (rank-2 matmul trick)
    expT[j, i] = exp(Prelu_0.1(pre) + posT)
    U[i, d], den[i] = expT.T @ [Wh0 | 1] (ones column gives the softmax denom)
Host epilogue:  out = ELU(U/den + b1)
    (exact: softmax @ (Wh0 + 1 b1^T) = (exp@Wh0)/den + b1 since exp@1 = den)

Heavy matmuls run in float32r (TF32-like, 1 cyc/row). x/pos/U I/O can run
in bf16 (flags below) to cut HBM traffic; den stays fp32.
"""
import sys

sys.path.insert(0, "/opt/trn_rl_repo")

import numpy as np
import ml_dtypes

import bass_rust as _bass_rust
import concourse.bass as bass
import concourse.bacc as bacc
import concourse.tile as tile_mod
from concourse import mybir
from concourse.bass_utils import run_bass_kernel_spmd

# ---------------------------------------------------------------------------
# Workaround: this container's walrus rejects >1 sync wait on a tail Drain.
# Split the TileContext tail-drain waits across one Drain per producing proc.
def _split_drain_and_barrier(self, tick_clock, wait_clock):
    gc = list(tick_clock.global_clock)
    for proc, sem in sorted(self.sems.allocated().items()):
        if gc[proc] <= 0:
            continue
        vec = [0] * len(gc)
        vec[proc] = gc[proc]
        drain_inst = self.nc.sync.drain()
        wait_clock.add_sem_waits(
            drain_inst.ins,
            _bass_rust.ScopedClock({None: _bass_rust.VectorClock(vec)}),
        )
    self.nc.sync.drain()
    self.nc.all_engine_barrier()
    assert self.sems is not None
    popped = self.nc._tile_sem_poison_stack.pop()
    assert popped is self._sem_poison
    self.nc.clear_and_free_semaphores(list(self.sems.allocated().values()))
    self.nc.all_engine_barrier()


tile_mod.TileContext._drain_and_barrier = _split_drain_and_barrier
# ---------------------------------------------------------------------------

F32 = mybir.dt.float32
F32R = mybir.dt.float32r
BF16 = mybir.dt.bfloat16
AF = mybir.ActivationFunctionType
ALU = mybir.AluOpType

B, S, D = 512, 256, 256
NCORES = 8
BLOC = B // NCORES          # 64 batches per core
G = BLOC // 2               # 32 groups of 2 batches
NEG_SLOPE = 0.1
NAUG = 258                  # Wh cols + ones col + zero pad (matmul N must be even)
NBUF = 3                    # manual buffering depth for A/B/whaug statics

X_BF16 = True               # ship x/W1/u2 in bf16 (Wh & e matmuls in bf16)
POS_BF16 = True             # ship pos in bf16 (added to fp32 logits on device)
OUT_BF16 = True             # ship U in bf16 (den stays fp32)

_PROG = None


def _build_program():
    xdt = BF16 if X_BF16 else F32
    xmmdt = BF16 if X_BF16 else F32R
    pdt = BF16 if POS_BF16 else F32
    odt = BF16 if OUT_BF16 else F32

    assert X_BF16 == POS_BF16, "merged xpos DMA needs matching dtypes"
    nc = bacc.Bacc()
    xpos = nc.declare_dram_parameter("xpos", [G, 128, 2, 2, 2, 256], xdt, isOutput=False)
    w1T = nc.declare_dram_parameter("w1T", [128, 2, 256], xdt, isOutput=False)
    u2 = nc.declare_dram_parameter("u2", [128, 2, 2], xdt, isOutput=False)
    cst = nc.declare_dram_parameter("cst", [2, 1], F32, isOutput=False)
    init = nc.declare_dram_parameter("init", [128, 1032], F32, isOutput=False)
    out_r = nc.declare_dram_parameter("out_r", [G, 128, 2, 2, 256], odt, isOutput=True)
    den_r = nc.declare_dram_parameter("den_r", [128, G, 2, 2], F32, isOutput=True)

    def mmcast(ap):
        return ap if X_BF16 else ap.bitcast(F32R)

    with tile_mod.TileContext(nc) as tc:
        with (
            tc.tile_pool(name="consts", bufs=1) as consts,
            tc.tile_pool(name="xp", bufs=6) as xp,
            tc.tile_pool(name="s1p", bufs=4) as s1p,
            tc.tile_pool(name="s2p", bufs=4) as s2p,
            tc.tile_pool(name="exq", bufs=4) as exq,
            tc.tile_pool(name="outp", bufs=6) as outp,
            tc.tile_pool(name="mm", bufs=2, space="PSUM") as mmp,
            tc.tile_pool(name="prep", bufs=2, space="PSUM") as prep,
        ):
            prefetch = {}
            xpgp0 = xp.tile([128, 2, 2, 2, 256], xmmdt, tag="xg", name="xpgpre0")
            nc.sync.dma_start(out=xpgp0[:], in_=mmcast(xpos.ap()[0]))
            prefetch[0] = xpgp0
            w1s = consts.tile([128, 2, 256], xmmdt)
            u2s = consts.tile([128, 2, 2], xmmdt)
            cs = consts.tile([2, 1], F32)
            nc.sync.dma_start(out=w1s[:], in_=mmcast(w1T.ap()[:]))
            nc.sync.dma_start(out=u2s[:], in_=mmcast(u2.ap()[:]))
            nc.sync.dma_start(out=cs[:], in_=cst.ap()[:])

            # Static rank-2 operand tiles (NBUF-buffered manually), K = 34:
            #   A  [34, b2, 128 j-cols]: rows (er, el*, 1..1, 0, 1)
            #   Bt [34, b2, 256 i-cols]: rows (1, 0..0, er+c*, el'+c)
            #   (* = dead rows: A row1 pairs Bt row1 = 0; Bt row32 pairs A row32 = 0.
            #    Copies write 2 rows at 32-aligned bases; HW requires base 0/32/64.)
            # pre = A.T @ Bt = er[j] + el'[i] + c
            A_t = [consts.tile([34, 2, 256], F32R, tag=f"A{i}", name=f"A{i}") for i in range(NBUF)]
            B_t = [consts.tile([34, 2, 256], F32R, tag=f"B{i}", name=f"B{i}") for i in range(NBUF)]
            wha = [consts.tile([128, 4, NAUG], F32R, tag=f"wha{i}", name=f"wha{i}") for i in range(NBUF)]
            def _init_statics(i):
                nc.sync.dma_start(out=A_t[i][:], in_=init.ap()[0:34, 0:512].bitcast(F32R))
                nc.sync.dma_start(out=B_t[i][:], in_=init.ap()[0:34, 512:1024].bitcast(F32R))
                nc.sync.dma_start(
                    out=wha[i][:, :, 256:NAUG], in_=init.ap()[:, 1024:1032].bitcast(F32R)
                )
            _init_statics(0)
            xpgp1 = xp.tile([128, 2, 2, 2, 256], xmmdt, tag="xg", name="xpgpre1")
            nc.sync.dma_start(out=xpgp1[:], in_=mmcast(xpos.ap()[1]))
            prefetch[1] = xpgp1
            for i in range(1, NBUF):
                _init_statics(i)

            denall = consts.tile([128, G, 2, 2], F32)

            # ---- software-pipelined group loop (1-group skew) ----------------
            # stage1(g): loads + all PE front work (Wh, e, rank-2) + copies
            # stage2(g): lrelu
            # stage3(g): posadd, exp, agg matmuls, output copies + DMA
            # Emission order s1(g), s2(g), s3(g-1) lets ACT run lrelu(g) while
            # POOL does posadd(g-1), hiding the ACT->POOL->ACT ping-pong.
            xpgs, preps, s1s, whs = {}, {}, {}, {}

            def stage1(g):
                if g in prefetch:
                    xpg = prefetch[g]
                else:
                    xpg = xp.tile([128, 2, 2, 2, 256], xmmdt, tag="xg", name=f"xpg{g}")
                    nc.sync.dma_start(out=xpg[:], in_=mmcast(xpos.ap()[g]))
                xpgs[g] = xpg
                xg = xpg[:, 0]
                A, Bt, whaug = A_t[g % NBUF], B_t[g % NBUF], wha[g % NBUF]

                wh_ps = mmp.tile([128, 4, 256], F32, tag="mm", name=f"wh{g}")
                whs[g] = wh_ps
                for b2 in range(2):
                    for st in range(2):
                        for k in range(2):
                            nc.tensor.matmul(
                                wh_ps[:, b2 * 2 + st, :],
                                xg[:, k, b2, bass.ts(st, 128)],
                                w1s[:, k, :],
                                start=(k == 0),
                                stop=(k == 1),
                            )
                pre_ps = prep.tile([128, 4, 256], F32, tag="pre", name=f"pre{g}")
                preps[g] = pre_ps
                for k in range(2):
                    nc.tensor.matmul(
                        pre_ps[0:2, 0:2, :],
                        u2s[:, k, :],
                        xg[:, k, :, :],
                        start=(k == 0),
                        stop=(k == 1),
                    )
                nc.vector.tensor_copy(A[0:2, :, :], pre_ps[0:2, 0:2, :])
                nc.gpsimd.tensor_scalar(
                    Bt[32:34, :, :], A[0:2, :, :], cs[0:2, 0:1], None, ALU.add
                )
                for b2 in range(2):
                    for jt in range(2):
                        nc.tensor.matmul(
                            pre_ps[:, b2 * 2 + jt, :],
                            A[:, b2, bass.ts(jt, 128)],
                            Bt[:, b2, :],
                            start=True,
                            stop=True,
                        )
                nc.vector.tensor_copy(whaug[:, :, 0:256], wh_ps[:, :, :])

            def stage2(g):
                s1 = s1p.tile([128, 4, 256], F32, tag="s1", name=f"s1_{g}")
                nc.scalar.activation(s1[:], preps[g][:, :, :], AF.Prelu, alpha=NEG_SLOPE)
                s1s[g] = s1

            def stage3(g):
                whaug = wha[g % NBUF]
                posg = xpgs[g][:, 1]
                s2 = s2p.tile([128, 4, 256], F32, tag="s2", name=f"s2_{g}")
                nc.gpsimd.tensor_tensor(s2[:], s1s[g][:], posg[:], op=ALU.add)
                expg = exq.tile([128, 4, 256], F32R, tag="expg", name=f"ex{g}")
                nc.scalar.activation(expg[:], s2[:], AF.Exp)

                outsb = outp.tile([128, 2, 2, 256], odt, tag="outsb", name=f"osb{g}")
                for b2 in range(2):
                    agg_ps = mmp.tile([128, 2, 512], F32, tag="mm", name=f"agg{g}_{b2}")
                    for it in range(2):
                        for jt in range(2):
                            nc.tensor.matmul(
                                agg_ps[:, it, 0:NAUG],
                                expg[:, b2 * 2 + jt, bass.ts(it, 128)],
                                whaug[:, b2 * 2 + jt, :],
                                start=(jt == 0),
                                stop=(jt == 1),
                            )
                    if b2 == (g % 2):
                        nc.scalar.copy(outsb[:, b2, :, :], agg_ps[:, :, 0:256])
                    else:
                        nc.vector.tensor_copy(outsb[:, b2, :, :], agg_ps[:, :, 0:256])
                    nc.vector.tensor_copy(denall[:, g, b2, :], agg_ps[:, :, 256])
                nc.sync.dma_start(out=out_r.ap()[g], in_=outsb[:])
                del xpgs[g], preps[g], s1s[g], whs[g]

            for g in range(G + 1):
                if g < G:
                    stage1(g)
                    stage2(g)
                if g > 0:
                    stage3(g - 1)
            nc.sync.dma_start(out=den_r.ap()[:], in_=denall[:])

    nc.finalize()
    return nc


def _get_prog():
    global _PROG
    if _PROG is None:
        _PROG = _build_program()
    return _PROG


def _host_inputs(x, pos_matrix, W1, b1, a_w, a_b):
    """Build per-core input maps."""
    x = np.asarray(x, dtype=np.float32)
    pos = np.asarray(pos_matrix, dtype=np.float32)[..., 0]  # [B, i, j]
    W1 = np.asarray(W1, dtype=np.float32)
    b1 = np.asarray(b1, dtype=np.float32)
    a_w = np.asarray(a_w, dtype=np.float32)
    a_b = np.asarray(a_b, dtype=np.float32)

    xdt = ml_dtypes.bfloat16 if X_BF16 else np.float32
    pdt = ml_dtypes.bfloat16 if POS_BF16 else np.float32

    a_l, a_r = a_w[:D], a_w[D:]
    u_l = W1.T @ a_l
    u_r = W1.T @ a_r
    c = np.float32(b1 @ a_l + b1 @ a_r + a_b[0])

    w1T_v = np.ascontiguousarray(W1.T.reshape(2, 128, 256).transpose(1, 0, 2)).astype(xdt)
    u2_v = np.ascontiguousarray(
        np.stack([u_r.reshape(2, 128), u_l.reshape(2, 128)], axis=-1).transpose(1, 0, 2)
    ).astype(xdt)
    cst_v = np.array([[c], [c]], dtype=np.float32)
    init_v = np.zeros((128, 1032), dtype=np.float32)
    init_v[0:34, 0:512] = 1.0              # A: ones ...
    init_v[32, 0:512] = 0.0                # ... except row 32 (kills er+c in B row32)
    init_v[0, 512:1024] = 1.0              # B: row0 ones, rest zero
    init_v[:, 1024:1032] = np.tile(np.array([1.0, 0.0], np.float32), 4)  # whaug cols

    in_maps = []
    for cix in range(NCORES):
        sl = slice(cix * BLOC, (cix + 1) * BLOC)
        xl = x[sl]                          # [64, s, d]
        # xT[g, p, k, b2, s] = xl[2g+b2, s, 128k+p]
        xT_v = xl.reshape(G, 2, S, 2, 128).transpose(0, 4, 3, 1, 2)
        pl = pos[sl]                        # [64, i, j]
        # posT[g, p, b2, jt, i] = pl[2g+b2, i, 128jt+p]
        posT_v = pl.reshape(G, 2, S, 2, 128).transpose(0, 4, 1, 3, 2)
        xpos_v = np.empty((G, 128, 2, 2, 2, 256), dtype=xdt)
        xpos_v[:, :, 0] = xT_v
        xpos_v[:, :, 1] = posT_v
        in_maps.append(
            {
                "xpos": xpos_v,
                "w1T": w1T_v,
                "u2": u2_v,
                "cst": cst_v,
                "init": init_v,
            }
        )
    return in_maps, b1


def _host_epilogue(results, b1):
    out = np.empty((B, S, D), dtype=np.float32)
    for cix in range(NCORES):
        U = np.asarray(results[cix]["out_r"], dtype=np.float32)   # [G, p, b2, it, d]
        den = np.asarray(results[cix]["den_r"], dtype=np.float32) # [p, G, b2, it]
        den = den.transpose(1, 0, 2, 3)                            # [G, p, b2, it]
        a = U / den[..., None] + b1
        # [G, p, b2, it, d] -> [G, b2, it, p, d] -> [64, 256, 256]
        a = a.transpose(0, 2, 3, 1, 4).reshape(BLOC, S, D)
        out[cix * BLOC : (cix + 1) * BLOC] = np.where(a > 0, a, np.expm1(a))
    return out


def kernel(x, pos_matrix, W1, b1, a_w, a_b):
    import time as _time

    nc = _get_prog()
    in_maps, b1v = _host_inputs(x, pos_matrix, W1, b1, a_w, a_b)
    last_exc = None
    for attempt in range(3):
        try:
            res = run_bass_kernel_spmd(nc, in_maps, core_ids=list(range(NCORES)))
            return _host_epilogue(res.results, b1v)
        except Exception as e:  # transient NRT/axon device errors recover on retry
            last_exc = e
            _time.sleep(10 * (attempt + 1))
    raise last_exc


# revision 21
# speedup vs baseline: 918.4951x; 1.0017x over previous
"""GAT layer kernel for Trainium2, 8 NeuronCores, data-parallel over batch.

Math (per batch b):
    Wh = x @ W1.T + b1
    e_l = Wh @ a_w[:D]; e_r = Wh @ a_w[D:]
    s = LeakyReLU_{0.1}(e_l[i] + e_r[j] + a_b) + pos[b, i, j]
    w = softmax_j(s);  out = ELU(w @ Wh)

Device computes, per batch (in transposed [j, i] layout):
    Wh0 = x @ W1.T                       (no bias; bias folds out, see below)
    er[j] = x @ (W1.T @ a_r)             el'[i] = x @ (W1.T @ a_l) + c
        where c = b1@a_l + b1@a_r + a_b  (so el' + er == e_l + e_r + a_b)
    pre[j, i] = er[j] + el'[i]           (rank-2 matmul trick)
    expT[j, i] = exp(Prelu_0.1(pre) + posT)
    U[i, d], den[i] = expT.T @ [Wh0 | 1] (ones column gives the softmax denom)
Host epilogue:  out = ELU(U/den + b1)
    (exact: softmax @ (Wh0 + 1 b1^T) = (exp@Wh0)/den + b1 since exp@1 = den)

Heavy matmuls run in float32r (TF32-like, 1 cyc/row). x/pos/U I/O can run
in bf16 (flags below) to cut HBM traffic; den stays fp32.
"""
import sys

sys.path.insert(0, "/opt/trn_rl_repo")

import numpy as np
import ml_dtypes

import bass_rust as _bass_rust
import concourse.bass as bass
import concourse.bacc as bacc
import concourse.tile as tile_mod
from concourse import mybir
from concourse.bass_utils import run_bass_kernel_spmd

# ---------------------------------------------------------------------------
# Workaround: this container's walrus rejects >1 sync wait on a tail Drain.
# Split the TileContext tail-drain waits across one Drain per producing proc.
def _split_drain_and_barrier(self, tick_clock, wait_clock):
    gc = list(tick_clock.global_clock)
    for proc, sem in sorted(self.sems.allocated().items()):
        if gc[proc] <= 0:
            continue
        vec = [0] * len(gc)
        vec[proc] = gc[proc]
        drain_inst = self.nc.sync.drain()
        wait_clock.add_sem_waits(
            drain_inst.ins,
            _bass_rust.ScopedClock({None: _bass_rust.VectorClock(vec)}),
        )
    self.nc.sync.drain()
    self.nc.all_engine_barrier()
    assert self.sems is not None
    popped = self.nc._tile_sem_poison_stack.pop()
    assert popped is self._sem_poison
    self.nc.clear_and_free_semaphores(list(self.sems.allocated().values()))
    self.nc.all_engine_barrier()


tile_mod.TileContext._drain_and_barrier = _split_drain_and_barrier
# ---------------------------------------------------------------------------

F32 = mybir.dt.float32
F32R = mybir.dt.float32r
BF16 = mybir.dt.bfloat16
AF = mybir.ActivationFunctionType
ALU = mybir.AluOpType

B, S, D = 512, 256, 256
NCORES = 8
BLOC = B // NCORES          # 64 batches per core
G = BLOC // 2               # 32 groups of 2 batches
NEG_SLOPE = 0.1
NAUG = 258                  # Wh cols + ones col + zero pad (matmul N must be even)
NBUF = 3                    # manual buffering depth for A/B/whaug statics

X_BF16 = True               # ship x/W1/u2 in bf16 (Wh & e matmuls in bf16)
POS_BF16 = True             # ship pos in bf16 (added to fp32 logits on device)
OUT_BF16 = True             # ship U in bf16 (den stays fp32)

_PROG = None


def _build_program():
    xdt = BF16 if X_BF16 else F32
    xmmdt = BF16 if X_BF16 else F32R
    pdt = BF16 if POS_BF16 else F32
    odt = BF16 if OUT_BF16 else F32

    assert X_BF16 == POS_BF16, "merged xpos DMA needs matching dtypes"
    nc = bacc.Bacc()
    xpos = nc.declare_dram_parameter("xpos", [G, 128, 2, 2, 2, 256], xdt, isOutput=False)
    w1T = nc.declare_dram_parameter("w1T", [128, 2, 256], xdt, isOutput=False)
    u2 = nc.declare_dram_parameter("u2", [128, 2, 2], xdt, isOutput=False)
    cst = nc.declare_dram_parameter("cst", [2, 1], F32, isOutput=False)
    init = nc.declare_dram_parameter("init", [128, 1032], F32, isOutput=False)
    out_r = nc.declare_dram_parameter("out_r", [G, 128, 2, 2, 256], odt, isOutput=True)
    den_r = nc.declare_dram_parameter("den_r", [128, G, 2, 2], F32, isOutput=True)

    def mmcast(ap):
        return ap if X_BF16 else ap.bitcast(F32R)

    with tile_mod.TileContext(nc) as tc:
        with (
            tc.tile_pool(name="consts", bufs=1) as consts,
            tc.tile_pool(name="xp", bufs=6) as xp,
            tc.tile_pool(name="s1p", bufs=4) as s1p,
            tc.tile_pool(name="s2p", bufs=4) as s2p,
            tc.tile_pool(name="exq", bufs=4) as exq,
            tc.tile_pool(name="outp", bufs=6) as outp,
            tc.tile_pool(name="mm", bufs=2, space="PSUM") as mmp,
            tc.tile_pool(name="prep", bufs=2, space="PSUM") as prep,
        ):
            prefetch = {}
            xpgp0 = xp.tile([128, 2, 2, 2, 256], xmmdt, tag="xg", name="xpgpre0")
            nc.sync.dma_start(out=xpgp0[:], in_=mmcast(xpos.ap()[0]))
            prefetch[0] = xpgp0
            w1s = consts.tile([128, 2, 256], xmmdt)
            u2s = consts.tile([128, 2, 2], xmmdt)
            cs = consts.tile([2, 1], F32)
            nc.sync.dma_start(out=w1s[:], in_=mmcast(w1T.ap()[:]))
            nc.sync.dma_start(out=u2s[:], in_=mmcast(u2.ap()[:]))
            nc.sync.dma_start(out=cs[:], in_=cst.ap()[:])

            # Static rank-2 operand tiles (NBUF-buffered manually), K = 34:
            #   A  [34, b2, 128 j-cols]: rows (er, el*, 1..1, 0, 1)
            #   Bt [34, b2, 256 i-cols]: rows (1, 0..0, er+c*, el'+c)
            #   (* = dead rows: A row1 pairs Bt row1 = 0; Bt row32 pairs A row32 = 0.
            #    Copies write 2 rows at 32-aligned bases; HW requires base 0/32/64.)
            # pre = A.T @ Bt = er[j] + el'[i] + c
            A_t = [consts.tile([34, 2, 256], F32R, tag=f"A{i}", name=f"A{i}") for i in range(NBUF)]
            B_t = [consts.tile([34, 2, 256], F32R, tag=f"B{i}", name=f"B{i}") for i in range(NBUF)]
            wha = [consts.tile([128, 4, NAUG], F32R, tag=f"wha{i}", name=f"wha{i}") for i in range(NBUF)]
            def _init_statics(i):
                nc.sync.dma_start(out=A_t[i][:], in_=init.ap()[0:34, 0:512].bitcast(F32R))
                nc.sync.dma_start(out=B_t[i][:], in_=init.ap()[0:34, 512:1024].bitcast(F32R))
                nc.sync.dma_start(
                    out=wha[i][:, :, 256:NAUG], in_=init.ap()[:, 1024:1032].bitcast(F32R)
                )
            _init_statics(0)
            xpgp1 = xp.tile([128, 2, 2, 2, 256], xmmdt, tag="xg", name="xpgpre1")
            nc.sync.dma_start(out=xpgp1[:], in_=mmcast(xpos.ap()[1]))
            prefetch[1] = xpgp1
            for i in range(1, NBUF):
                _init_statics(i)

            denall = consts.tile([128, G, 2, 2], F32)

            # ---- software-pipelined group loop (1-group skew) ----------------
            # stage1(g): loads + all PE front work (Wh, e, rank-2) + copies
            # stage2(g): lrelu
            # stage3(g): posadd, exp, agg matmuls, output copies + DMA
            # Emission order s1(g), s2(g), s3(g-1) lets ACT run lrelu(g) while
            # POOL does posadd(g-1), hiding the ACT->POOL->ACT ping-pong.
            xpgs, preps, s1s, whs = {}, {}, {}, {}

            def stage1(g):
                if g in prefetch:
                    xpg = prefetch[g]
                else:
                    xpg = xp.tile([128, 2, 2, 2, 256], xmmdt, tag="xg", name=f"xpg{g}")
                    nc.sync.dma_start(out=xpg[:], in_=mmcast(xpos.ap()[g]))
                xpgs[g] = xpg
                xg = xpg[:, 0]
                A, Bt, whaug = A_t[g % NBUF], B_t[g % NBUF], wha[g % NBUF]

                wh_ps = mmp.tile([128, 4, 256], F32, tag="mm", name=f"wh{g}")
                whs[g] = wh_ps
                for b2 in range(2):
                    for st in range(2):
                        for k in range(2):
                            nc.tensor.matmul(
                                wh_ps[:, b2 * 2 + st, :],
                                xg[:, k, b2, bass.ts(st, 128)],
                                w1s[:, k, :],
                                start=(k == 0),
                                stop=(k == 1),
                            )
                pre_ps = prep.tile([128, 4, 256], F32, tag="pre", name=f"pre{g}")
                preps[g] = pre_ps
                for k in range(2):
                    nc.tensor.matmul(
                        pre_ps[0:2, 0:2, :],
                        u2s[:, k, :],
                        xg[:, k, :, :],
                        start=(k == 0),
                        stop=(k == 1),
                    )
                nc.vector.tensor_copy(A[0:2, :, :], pre_ps[0:2, 0:2, :])
                nc.gpsimd.tensor_scalar(
                    Bt[32:34, :, :], A[0:2, :, :], cs[0:2, 0:1], None, ALU.add
                )
                for b2 in range(2):
                    for jt in range(2):
                        nc.tensor.matmul(
                            pre_ps[:, b2 * 2 + jt, :],
                            A[:, b2, bass.ts(jt, 128)],
                            Bt[:, b2, :],
                            start=True,
                            stop=True,
                        )
                nc.vector.tensor_copy(whaug[:, :, 0:256], wh_ps[:, :, :])

            def stage2(g):
                s1 = s1p.tile([128, 4, 256], F32, tag="s1", name=f"s1_{g}")
                nc.scalar.activation(s1[:], preps[g][:, :, :], AF.Prelu, alpha=NEG_SLOPE)
                s1s[g] = s1

            def stage3(g):
                whaug = wha[g % NBUF]
                posg = xpgs[g][:, 1]
                s2 = s2p.tile([128, 4, 256], F32, tag="s2", name=f"s2_{g}")
                nc.gpsimd.tensor_tensor(s2[:], s1s[g][:], posg[:], op=ALU.add)
                expg = exq.tile([128, 4, 256], F32R, tag="expg", name=f"ex{g}")
                nc.scalar.activation(expg[:], s2[:], AF.Exp)

                outsb = outp.tile([128, 2, 2, 256], odt, tag="outsb", name=f"osb{g}")
                for b2 in range(2):
                    agg_ps = mmp.tile([128, 2, 512], F32, tag="mm", name=f"agg{g}_{b2}")
                    for it in range(2):
                        for jt in range(2):
                            nc.tensor.matmul(
                                agg_ps[:, it, 0:NAUG],
                                expg[:, b2 * 2 + jt, bass.ts(it, 128)],
                                whaug[:, b2 * 2 + jt, :],
                                start=(jt == 0),
                                stop=(jt == 1),
                            )
                    if b2 == (g % 2):
                        nc.scalar.copy(outsb[:, b2, :, :], agg_ps[:, :, 0:256])
                    else:
                        nc.vector.tensor_copy(outsb[:, b2, :, :], agg_ps[:, :, 0:256])
                    nc.vector.tensor_copy(denall[:, g, b2, :], agg_ps[:, :, 256])
                nc.sync.dma_start(out=out_r.ap()[g], in_=outsb[:])
                del xpgs[g], preps[g], s1s[g], whs[g]

            for g in range(G + 1):
                if g < G:
                    stage1(g)
                if g > 0:
                    stage3(g - 1)
                if g < G:
                    stage2(g)
            nc.sync.dma_start(out=den_r.ap()[:], in_=denall[:])

    nc.finalize()
    return nc


def _get_prog():
    global _PROG
    if _PROG is None:
        _PROG = _build_program()
    return _PROG


def _host_inputs(x, pos_matrix, W1, b1, a_w, a_b):
    """Build per-core input maps."""
    x = np.asarray(x, dtype=np.float32)
    pos = np.asarray(pos_matrix, dtype=np.float32)[..., 0]  # [B, i, j]
    W1 = np.asarray(W1, dtype=np.float32)
    b1 = np.asarray(b1, dtype=np.float32)
    a_w = np.asarray(a_w, dtype=np.float32)
    a_b = np.asarray(a_b, dtype=np.float32)

    xdt = ml_dtypes.bfloat16 if X_BF16 else np.float32
    pdt = ml_dtypes.bfloat16 if POS_BF16 else np.float32

    a_l, a_r = a_w[:D], a_w[D:]
    u_l = W1.T @ a_l
    u_r = W1.T @ a_r
    c = np.float32(b1 @ a_l + b1 @ a_r + a_b[0])

    w1T_v = np.ascontiguousarray(W1.T.reshape(2, 128, 256).transpose(1, 0, 2)).astype(xdt)
    u2_v = np.ascontiguousarray(
        np.stack([u_r.reshape(2, 128), u_l.reshape(2, 128)], axis=-1).transpose(1, 0, 2)
    ).astype(xdt)
    cst_v = np.array([[c], [c]], dtype=np.float32)
    init_v = np.zeros((128, 1032), dtype=np.float32)
    init_v[0:34, 0:512] = 1.0              # A: ones ...
    init_v[32, 0:512] = 0.0                # ... except row 32 (kills er+c in B row32)
    init_v[0, 512:1024] = 1.0              # B: row0 ones, rest zero
    init_v[:, 1024:1032] = np.tile(np.array([1.0, 0.0], np.float32), 4)  # whaug cols

    in_maps = []
    for cix in range(NCORES):
        sl = slice(cix * BLOC, (cix + 1) * BLOC)
        xl = x[sl]                          # [64, s, d]
        # xT[g, p, k, b2, s] = xl[2g+b2, s, 128k+p]
        xT_v = xl.reshape(G, 2, S, 2, 128).transpose(0, 4, 3, 1, 2)
        pl = pos[sl]                        # [64, i, j]
        # posT[g, p, b2, jt, i] = pl[2g+b2, i, 128jt+p]
        posT_v = pl.reshape(G, 2, S, 2, 128).transpose(0, 4, 1, 3, 2)
        xpos_v = np.empty((G, 128, 2, 2, 2, 256), dtype=xdt)
        xpos_v[:, :, 0] = xT_v
        xpos_v[:, :, 1] = posT_v
        in_maps.append(
            {
                "xpos": xpos_v,
                "w1T": w1T_v,
                "u2": u2_v,
                "cst": cst_v,
                "init": init_v,
            }
        )
    return in_maps, b1


def _host_epilogue(results, b1):
    out = np.empty((B, S, D), dtype=np.float32)
    for cix in range(NCORES):
        U = np.asarray(results[cix]["out_r"], dtype=np.float32)   # [G, p, b2, it, d]
        den = np.asarray(results[cix]["den_r"], dtype=np.float32) # [p, G, b2, it]
        den = den.transpose(1, 0, 2, 3)                            # [G, p, b2, it]
        a = U / den[..., None] + b1
        # [G, p, b2, it, d] -> [G, b2, it, p, d] -> [64, 256, 256]
        a = a.transpose(0, 2, 3, 1, 4).reshape(BLOC, S, D)
        out[cix * BLOC : (cix + 1) * BLOC] = np.where(a > 0, a, np.expm1(a))
    return out


def kernel(x, pos_matrix, W1, b1, a_w, a_b):
    import time as _time

    nc = _get_prog()
    in_maps, b1v = _host_inputs(x, pos_matrix, W1, b1, a_w, a_b)
    last_exc = None
    for attempt in range(3):
        try:
            res = run_bass_kernel_spmd(nc, in_maps, core_ids=list(range(NCORES)))
            return _host_epilogue(res.results, b1v)
        except Exception as e:  # transient NRT/axon device errors recover on retry
            last_exc = e
            _time.sleep(10 * (attempt + 1))
    raise last_exc


# revision 24
# speedup vs baseline: 922.5897x; 1.0045x over previous
"""GAT layer kernel for Trainium2, 8 NeuronCores, data-parallel over batch.

Math (per batch b):
    Wh = x @ W1.T + b1
    e_l = Wh @ a_w[:D]; e_r = Wh @ a_w[D:]
    s = LeakyReLU_{0.1}(e_l[i] + e_r[j] + a_b) + pos[b, i, j]
    w = softmax_j(s);  out = ELU(w @ Wh)

Device computes, per batch (in transposed [j, i] layout):
    Wh0 = x @ W1.T                       (no bias; bias folds out, see below)
    er[j] = x @ (W1.T @ a_r)             el'[i] = x @ (W1.T @ a_l) + c
        where c = b1@a_l + b1@a_r + a_b  (so el' + er == e_l + e_r + a_b)
    pre[j, i] = er[j] + el'[i]           (rank-2 matmul trick)
    expT[j, i] = exp(Prelu_0.1(pre) + posT)
    U[i, d], den[i] = expT.T @ [Wh0 | 1] (ones column gives the softmax denom)
Host epilogue:  out = ELU(U/den + b1)
    (exact: softmax @ (Wh0 + 1 b1^T) = (exp@Wh0)/den + b1 since exp@1 = den)

Heavy matmuls run in float32r (TF32-like, 1 cyc/row). x/pos/U I/O can run
in bf16 (flags below) to cut HBM traffic; den stays fp32.
"""
import sys

sys.path.insert(0, "/opt/trn_rl_repo")

import numpy as np
import ml_dtypes

import bass_rust as _bass_rust
import concourse.bass as bass
import concourse.bacc as bacc
import concourse.tile as tile_mod
from concourse import mybir
from concourse.bass_utils import run_bass_kernel_spmd

# ---------------------------------------------------------------------------
# Workaround: this container's walrus rejects >1 sync wait on a tail Drain.
# Split the TileContext tail-drain waits across one Drain per producing proc.
def _split_drain_and_barrier(self, tick_clock, wait_clock):
    gc = list(tick_clock.global_clock)
    for proc, sem in sorted(self.sems.allocated().items()):
        if gc[proc] <= 0:
            continue
        vec = [0] * len(gc)
        vec[proc] = gc[proc]
        drain_inst = self.nc.sync.drain()
        wait_clock.add_sem_waits(
            drain_inst.ins,
            _bass_rust.ScopedClock({None: _bass_rust.VectorClock(vec)}),
        )
    self.nc.sync.drain()
    self.nc.all_engine_barrier()
    assert self.sems is not None
    popped = self.nc._tile_sem_poison_stack.pop()
    assert popped is self._sem_poison
    self.nc.clear_and_free_semaphores(list(self.sems.allocated().values()))
    self.nc.all_engine_barrier()


tile_mod.TileContext._drain_and_barrier = _split_drain_and_barrier
# ---------------------------------------------------------------------------

F32 = mybir.dt.float32
F32R = mybir.dt.float32r
BF16 = mybir.dt.bfloat16
AF = mybir.ActivationFunctionType
ALU = mybir.AluOpType

B, S, D = 512, 256, 256
NCORES = 8
BLOC = B // NCORES          # 64 batches per core
G = BLOC // 2               # 32 groups of 2 batches
NEG_SLOPE = 0.1
NAUG = 258                  # Wh cols + ones col + zero pad (matmul N must be even)
NBUF = 3                    # manual buffering depth for A/B/whaug statics

X_BF16 = True               # ship x/W1/u2 in bf16 (Wh & e matmuls in bf16)
POS_BF16 = True             # ship pos in bf16 (added to fp32 logits on device)
OUT_BF16 = True             # ship U in bf16 (den stays fp32)

_PROG = None


def _build_program():
    xdt = BF16 if X_BF16 else F32
    xmmdt = BF16 if X_BF16 else F32R
    pdt = BF16 if POS_BF16 else F32
    odt = BF16 if OUT_BF16 else F32

    assert X_BF16 == POS_BF16, "merged xpos DMA needs matching dtypes"
    nc = bacc.Bacc()
    xpos = nc.declare_dram_parameter("xpos", [G, 128, 2, 2, 2, 256], xdt, isOutput=False)
    w1T = nc.declare_dram_parameter("w1T", [128, 2, 256], xdt, isOutput=False)
    u2 = nc.declare_dram_parameter("u2", [128, 2, 2], xdt, isOutput=False)
    cst = nc.declare_dram_parameter("cst", [2, 1], F32, isOutput=False)
    init = nc.declare_dram_parameter("init", [128, 1032], F32, isOutput=False)
    out_r = nc.declare_dram_parameter("out_r", [G, 128, 2, 2, 256], odt, isOutput=True)
    den_r = nc.declare_dram_parameter("den_r", [128, G, 2, 2], F32, isOutput=True)

    def mmcast(ap):
        return ap if X_BF16 else ap.bitcast(F32R)

    with tile_mod.TileContext(nc) as tc:
        with (
            tc.tile_pool(name="consts", bufs=1) as consts,
            tc.tile_pool(name="xp", bufs=6) as xp,
            tc.tile_pool(name="s1p", bufs=4) as s1p,
            tc.tile_pool(name="s2p", bufs=4) as s2p,
            tc.tile_pool(name="exq", bufs=4) as exq,
            tc.tile_pool(name="outp", bufs=6) as outp,
            tc.tile_pool(name="mm", bufs=2, space="PSUM") as mmp,
            tc.tile_pool(name="prep", bufs=2, space="PSUM") as prep,
        ):
            prefetch = {}
            xpgp0 = xp.tile([128, 2, 2, 2, 256], xmmdt, tag="xg", name="xpgpre0")
            nc.sync.dma_start(out=xpgp0[:], in_=mmcast(xpos.ap()[0]))
            prefetch[0] = xpgp0
            w1s = consts.tile([128, 2, 256], xmmdt)
            u2s = consts.tile([128, 2, 2], xmmdt)
            cs = consts.tile([2, 1], F32)
            nc.sync.dma_start(out=w1s[:], in_=mmcast(w1T.ap()[:]))
            nc.sync.dma_start(out=u2s[:], in_=mmcast(u2.ap()[:]))
            nc.sync.dma_start(out=cs[:], in_=cst.ap()[:])

            # Static rank-2 operand tiles (NBUF-buffered manually), K = 34:
            #   A  [34, b2, 128 j-cols]: rows (er, el*, 1..1, 0, 1)
            #   Bt [34, b2, 256 i-cols]: rows (1, 0..0, er+c*, el'+c)
            #   (* = dead rows: A row1 pairs Bt row1 = 0; Bt row32 pairs A row32 = 0.
            #    Copies write 2 rows at 32-aligned bases; HW requires base 0/32/64.)
            # pre = A.T @ Bt = er[j] + el'[i] + c
            A_t = [consts.tile([34, 2, 256], F32R, tag=f"A{i}", name=f"A{i}") for i in range(NBUF)]
            B_t = [consts.tile([34, 2, 256], F32R, tag=f"B{i}", name=f"B{i}") for i in range(NBUF)]
            wha = [consts.tile([128, 4, NAUG], F32R, tag=f"wha{i}", name=f"wha{i}") for i in range(NBUF)]
            def _init_statics(i):
                nc.sync.dma_start(out=A_t[i][:], in_=init.ap()[0:34, 0:512].bitcast(F32R))
                nc.sync.dma_start(out=B_t[i][:], in_=init.ap()[0:34, 512:1024].bitcast(F32R))
                nc.sync.dma_start(
                    out=wha[i][:, :, 256:NAUG], in_=init.ap()[:, 1024:1032].bitcast(F32R)
                )
            _init_statics(0)
            xpgp1 = xp.tile([128, 2, 2, 2, 256], xmmdt, tag="xg", name="xpgpre1")
            nc.sync.dma_start(out=xpgp1[:], in_=mmcast(xpos.ap()[1]))
            prefetch[1] = xpgp1
            for i in range(1, NBUF):
                _init_statics(i)

            denall = consts.tile([128, G, 2, 2], F32)

            # ---- software-pipelined group loop (1-group skew) ----------------
            # stage1(g): loads + all PE front work (Wh, e, rank-2) + copies
            # stage2(g): lrelu
            # stage3(g): posadd, exp, agg matmuls, output copies + DMA
            # Emission order s1(g), s2(g), s3(g-1) lets ACT run lrelu(g) while
            # POOL does posadd(g-1), hiding the ACT->POOL->ACT ping-pong.
            xpgs, preps, s1s, whs = {}, {}, {}, {}

            def stage1(g):
                if g in prefetch:
                    xpg = prefetch[g]
                else:
                    xpg = xp.tile([128, 2, 2, 2, 256], xmmdt, tag="xg", name=f"xpg{g}")
                    nc.sync.dma_start(out=xpg[:], in_=mmcast(xpos.ap()[g]))
                xpgs[g] = xpg
                xg = xpg[:, 0]
                A, Bt, whaug = A_t[g % NBUF], B_t[g % NBUF], wha[g % NBUF]

                wh_ps = mmp.tile([128, 4, 256], F32, tag="mm", name=f"wh{g}")
                whs[g] = wh_ps
                for b2 in range(2):
                    for st in range(2):
                        for k in range(2):
                            nc.tensor.matmul(
                                wh_ps[:, b2 * 2 + st, :],
                                xg[:, k, b2, bass.ts(st, 128)],
                                w1s[:, k, :],
                                start=(k == 0),
                                stop=(k == 1),
                            )
                pre_ps = prep.tile([128, 4, 256], F32, tag="pre", name=f"pre{g}")
                preps[g] = pre_ps
                for k in range(2):
                    nc.tensor.matmul(
                        pre_ps[0:2, 0:2, :],
                        u2s[:, k, :],
                        xg[:, k, :, :],
                        start=(k == 0),
                        stop=(k == 1),
                    )
                nc.vector.tensor_copy(A[0:2, :, :], pre_ps[0:2, 0:2, :])
                nc.gpsimd.tensor_scalar(
                    Bt[32:34, :, :], A[0:2, :, :], cs[0:2, 0:1], None, ALU.add
                )
                for b2 in range(2):
                    for jt in range(2):
                        nc.tensor.matmul(
                            pre_ps[:, b2 * 2 + jt, :],
                            A[:, b2, bass.ts(jt, 128)],
                            Bt[:, b2, :],
                            start=True,
                            stop=True,
                        )
                nc.vector.tensor_copy(whaug[:, :, 0:256], wh_ps[:, :, :])

            def stage2(g):
                s1 = s1p.tile([128, 4, 256], F32, tag="s1", name=f"s1_{g}")
                nc.scalar.activation(s1[:], preps[g][:, :, :], AF.Prelu, alpha=NEG_SLOPE)
                s1s[g] = s1

            def stage3(g):
                whaug = wha[g % NBUF]
                posg = xpgs[g][:, 1]
                s2 = s2p.tile([128, 4, 256], F32, tag="s2", name=f"s2_{g}")
                nc.gpsimd.tensor_tensor(s2[:], s1s[g][:], posg[:], op=ALU.add)
                expg = exq.tile([128, 4, 256], F32R, tag="expg", name=f"ex{g}")
                nc.scalar.activation(expg[:], s2[:], AF.Exp)

                outsb = outp.tile([128, 2, 2, 256], odt, tag="outsb", name=f"osb{g}")
                for b2 in range(2):
                    agg_ps = mmp.tile([128, 2, 512], F32, tag="mm", name=f"agg{g}_{b2}")
                    for it in range(2):
                        for jt in range(2):
                            nc.tensor.matmul(
                                agg_ps[:, it, 0:NAUG],
                                expg[:, b2 * 2 + jt, bass.ts(it, 128)],
                                whaug[:, b2 * 2 + jt, :],
                                start=(jt == 0),
                                stop=(jt == 1),
                            )
                    if b2 == (g % 2):
                        nc.scalar.copy(outsb[:, b2, :, :], agg_ps[:, :, 0:256])
                    else:
                        nc.vector.tensor_copy(outsb[:, b2, :, :], agg_ps[:, :, 0:256])
                    nc.vector.tensor_copy(denall[:, g, b2, :], agg_ps[:, :, 256])
                nc.sync.dma_start(out=out_r.ap()[g], in_=outsb[:])
                del xpgs[g], preps[g], s1s[g], whs[g]

            stage1(0)
            stage1(1)
            stage2(0)
            for g in range(1, G):
                if g + 1 < G:
                    stage1(g + 1)
                stage3(g - 1)
                stage2(g)
            stage3(G - 1)
            nc.sync.dma_start(out=den_r.ap()[:], in_=denall[:])

    nc.finalize()
    return nc


def _get_prog():
    global _PROG
    if _PROG is None:
        _PROG = _build_program()
    return _PROG


def _host_inputs(x, pos_matrix, W1, b1, a_w, a_b):
    """Build per-core input maps."""
    x = np.asarray(x, dtype=np.float32)
    pos = np.asarray(pos_matrix, dtype=np.float32)[..., 0]  # [B, i, j]
    W1 = np.asarray(W1, dtype=np.float32)
    b1 = np.asarray(b1, dtype=np.float32)
    a_w = np.asarray(a_w, dtype=np.float32)
    a_b = np.asarray(a_b, dtype=np.float32)

    xdt = ml_dtypes.bfloat16 if X_BF16 else np.float32
    pdt = ml_dtypes.bfloat16 if POS_BF16 else np.float32

    a_l, a_r = a_w[:D], a_w[D:]
    u_l = W1.T @ a_l
    u_r = W1.T @ a_r
    c = np.float32(b1 @ a_l + b1 @ a_r + a_b[0])

    w1T_v = np.ascontiguousarray(W1.T.reshape(2, 128, 256).transpose(1, 0, 2)).astype(xdt)
    u2_v = np.ascontiguousarray(
        np.stack([u_r.reshape(2, 128), u_l.reshape(2, 128)], axis=-1).transpose(1, 0, 2)
    ).astype(xdt)
    cst_v = np.array([[c], [c]], dtype=np.float32)
    init_v = np.zeros((128, 1032), dtype=np.float32)
    init_v[0:34, 0:512] = 1.0              # A: ones ...
    init_v[32, 0:512] = 0.0                # ... except row 32 (kills er+c in B row32)
    init_v[0, 512:1024] = 1.0              # B: row0 ones, rest zero
    init_v[:, 1024:1032] = np.tile(np.array([1.0, 0.0], np.float32), 4)  # whaug cols

    in_maps = []
    for cix in range(NCORES):
        sl = slice(cix * BLOC, (cix + 1) * BLOC)
        xl = x[sl]                          # [64, s, d]
        # xT[g, p, k, b2, s] = xl[2g+b2, s, 128k+p]
        xT_v = xl.reshape(G, 2, S, 2, 128).transpose(0, 4, 3, 1, 2)
        pl = pos[sl]                        # [64, i, j]
        # posT[g, p, b2, jt, i] = pl[2g+b2, i, 128jt+p]
        posT_v = pl.reshape(G, 2, S, 2, 128).transpose(0, 4, 1, 3, 2)
        xpos_v = np.empty((G, 128, 2, 2, 2, 256), dtype=xdt)
        xpos_v[:, :, 0] = xT_v
        xpos_v[:, :, 1] = posT_v
        in_maps.append(
            {
                "xpos": xpos_v,
                "w1T": w1T_v,
                "u2": u2_v,
                "cst": cst_v,
                "init": init_v,
            }
        )
    return in_maps, b1


def _host_epilogue(results, b1):
    out = np.empty((B, S, D), dtype=np.float32)
    for cix in range(NCORES):
        U = np.asarray(results[cix]["out_r"], dtype=np.float32)   # [G, p, b2, it, d]
        den = np.asarray(results[cix]["den_r"], dtype=np.float32) # [p, G, b2, it]
        den = den.transpose(1, 0, 2, 3)                            # [G, p, b2, it]
        a = U / den[..., None] + b1
        # [G, p, b2, it, d] -> [G, b2, it, p, d] -> [64, 256, 256]
        a = a.transpose(0, 2, 3, 1, 4).reshape(BLOC, S, D)
        out[cix * BLOC : (cix + 1) * BLOC] = np.where(a > 0, a, np.expm1(a))
    return out


def kernel(x, pos_matrix, W1, b1, a_w, a_b):
    import time as _time

    nc = _get_prog()
    in_maps, b1v = _host_inputs(x, pos_matrix, W1, b1, a_w, a_b)
    last_exc = None
    for attempt in range(3):
        try:
            res = run_bass_kernel_spmd(nc, in_maps, core_ids=list(range(NCORES)))
            return _host_epilogue(res.results, b1v)
        except Exception as e:  # transient NRT/axon device errors recover on retry
            last_exc = e
            _time.sleep(10 * (attempt + 1))
    raise last_exc
